# revision 20
# baseline (speedup 1.0000x reference)
"""Multi-head attention forward (B=2, S=2048, H=2048, 16 heads) on 8 TRN2 NeuronCores.

Sharding: tensor-parallel over heads — 2 heads per core. Each core computes
Q/K/V projections for its 2 heads (full batch), attention, and a partial
output projection (its heads' columns of Wo); the host sums the 8 partial
outputs and adds the bias terms.

Device compute is bf16 with fp32 PSUM accumulation. Host pre-transposes
the activation matrix (X.T) and weight slices so the device never has to
transpose fp32 data (fp32 DMA transpose is unsupported).

Layout notes (matmul computes lhsT.T @ rhs, contracting the partition dim):
  - Q.T, K.T are computed as [head_dim, tokens] (d on partitions):
        lhsT = Wq.T tile [hid, d], rhs = X.T tile [hid, tokens]
  - V is computed natural [tokens, d]: lhsT = X.T tile, rhs = Wv.T tile
  - scores transposed S.T[k_tok, q] = (K.T tile).T @ Q.T  (contract d=128)
  - P.T = exp(SCALE * S.T + mask) via one scalar-engine activation
    (mask is per-key = per-partition, so it rides the activation bias)
  - ctx.T[d, q] = V_tile.T @ P.T (contract k_tok), accumulated over k tiles
  - softmax denominators via ones-vector matmul: [1,q] += ones.T @ P.T
  - out_partial[t, o] = (ctx.T tile).T @ Wo.T tile (contract local head dims)

bv/bo are folded on the host: rows of normalized P sum to 1, so
ctx = P@(V + bv) = P@V + bv, giving out += bv @ Wo.T + bo after the
cross-core reduction.
"""

import os

import numpy as np
import ml_dtypes

P = 128
HIDDEN = 2048
NUM_HEADS = 16
HEAD_DIM = 128
B, S = 2, 2048
T = B * S                     # 4096 tokens
N_CORES = 8
H_LOC = NUM_HEADS // N_CORES  # 2 heads per core
DLOC = H_LOC * HEAD_DIM       # 256
KO = HIDDEN // P              # 16 contraction tiles for the projections
CH = 8                        # token chunks for the projection phase
CHW = T // CH                 # 512 tokens per chunk
NKT = S // P                  # 16 key tiles per batch
NQ = S // 512                 # 4 query tiles (512 wide) per batch
SCALE = float(1.0 / np.sqrt(HEAD_DIM).astype(np.float32))

BF16NP = ml_dtypes.bfloat16

_CACHE = {}


def _split_multi_waits(nc):
    """Split instructions carrying >1 semaphore wait.

    This walrus build rejects any instruction with more than one sync wait
    ("Too many sync wait commands"), but Tile's wait assignment freely
    attaches several. Hoist all but the last wait onto same-engine NOPs
    inserted immediately before the instruction — each engine sequencer
    executes its queue in order, so blocking on a preceding NOP is
    equivalent to blocking on the instruction itself.
    """
    import bass_rust
    import concourse.mybir as mybir

    cnt = 0
    for f in nc.m.functions:
        for bb in f.blocks:
            out = []
            for inst in bb.instructions:
                si = inst.sync_info
                waits = list(si.on_wait) if si and si.on_wait else []
                if len(waits) > 1:
                    for w in waits[:-1]:
                        nop = mybir.InstNoOp(name=f"wsplit_{cnt}", ins=[], outs=[])
                        cnt += 1
                        nop.engine = inst.engine
                        nop.sync_info = bass_rust.SyncInfo(on_wait=[w], on_update=[])
                        out.append(nop)
                    inst.sync_info = bass_rust.SyncInfo(
                        on_wait=[waits[-1]], on_update=list(si.on_update or [])
                    )
                out.append(inst)
            bb.instructions[:] = out
    return cnt


def _build_nc(loop_k=None, **opts):
    """Build the kernel module.

    loop_k: if set, wrap the whole compute body in a For_i running it loop_k
    times — used only for benchmarking (slope timing); the graded kernel
    uses loop_k=None (straight-line body).
    opts: benchmark-only ablation switches (default: all off).
    """
    import concourse.bass as bass
    import concourse.mybir as mybir
    import concourse.tile as tile

    no_sums = opts.get("no_sums", False)
    no_phase3 = opts.get("no_phase3", False)
    no_out_dma = opts.get("no_out_dma", False)
    no_attn = opts.get("no_attn", False)
    xch_bufs = opts.get("xch_bufs", 3)
    pt_bufs = opts.get("pt_bufs", 4)
    norm2 = opts.get("norm2", False)        # sbuf->sbuf bcast + deferred norm
    interleave = opts.get("interleave", False)  # phase 2/3 interleaved per batch

    fp32 = mybir.dt.float32
    bf16 = mybir.dt.bfloat16

    nc = bass.Bass()

    xt_d = nc.dram_tensor("xt", [HIDDEN, T], bf16, kind="ExternalInput")
    wqt_d = nc.dram_tensor("wqt", [HIDDEN, DLOC], bf16, kind="ExternalInput")
    wkt_d = nc.dram_tensor("wkt", [HIDDEN, DLOC], bf16, kind="ExternalInput")
    wvt_d = nc.dram_tensor("wvt", [HIDDEN, DLOC], bf16, kind="ExternalInput")
    wot_d = nc.dram_tensor("wot", [DLOC, HIDDEN], bf16, kind="ExternalInput")
    bq_d = nc.dram_tensor("bq", [DLOC], fp32, kind="ExternalInput")
    bk_d = nc.dram_tensor("bk", [DLOC], fp32, kind="ExternalInput")
    mask_d = nc.dram_tensor("mask", [B, S], fp32, kind="ExternalInput")
    out_d = nc.dram_tensor("out", [T, HIDDEN], fp32, kind="ExternalOutput")

    xt_v = xt_d[:].rearrange("(ko p) t -> p ko t", p=P)
    wqt_v = wqt_d[:].rearrange("(ko p) d -> p ko d", p=P)
    wkt_v = wkt_d[:].rearrange("(ko p) d -> p ko d", p=P)
    wvt_v = wvt_d[:].rearrange("(ko p) d -> p ko d", p=P)
    wot_v = wot_d[:].rearrange("(h p) o -> p h o", p=P)
    bq_v = bq_d[:].rearrange("(h p) -> p h", p=P)
    bk_v = bk_d[:].rearrange("(h p) -> p h", p=P)
    mask_v = mask_d[:].rearrange("b (ko p) -> p b ko", p=P)

    with tile.TileContext(nc) as tc:
        with (
            tc.tile_pool(name="const", bufs=1) as const,
            tc.tile_pool(name="big", bufs=1) as big,
            tc.tile_pool(name="xch", bufs=xch_bufs) as xch,
            tc.tile_pool(name="ptp", bufs=pt_bufs) as ptp,
            tc.tile_pool(name="nrm", bufs=2) as nrm,
            tc.tile_pool(name="ob", bufs=3) as obp,
            tc.tile_pool(name="ps", bufs=8, space="PSUM") as psp,
            tc.tile_pool(name="dscr", bufs=4, space="DRAM") as dscr,
        ):
            Ident = mybir.ActivationFunctionType.Identity
            Exp = mybir.ActivationFunctionType.Exp

            def ps_tile():
                return psp.tile([P, 512], fp32, tag="ps", name="ps")

            # ---- resident constants -------------------------------------
            wq_sb = const.tile([P, KO, DLOC], bf16)
            wk_sb = const.tile([P, KO, DLOC], bf16)
            wv_sb = const.tile([P, KO, DLOC], bf16)
            wo_sb = const.tile([P, H_LOC, HIDDEN], bf16)
            bq_sb = const.tile([P, H_LOC], fp32)
            bk_sb = const.tile([P, H_LOC], fp32)
            mask_sb = const.tile([P, B, NKT], fp32)
            ones_sb = const.tile([P, 1], bf16)

            nc.sync.dma_start(wq_sb[:], wqt_v)
            nc.sync.dma_start(wk_sb[:], wkt_v)
            nc.sync.dma_start(wv_sb[:], wvt_v)
            nc.sync.dma_start(wo_sb[:], wot_v)
            nc.sync.dma_start(bq_sb[:], bq_v)
            nc.sync.dma_start(bk_sb[:], bk_v)
            nc.sync.dma_start(mask_sb[:], mask_v)
            nc.vector.memset(ones_sb[:], 1.0)

            # ---- big activation buffers ---------------------------------
            qt_sb = big.tile([P, H_LOC, T], bf16)   # Q.T  (d on partitions)
            kt_sb = big.tile([P, H_LOC, T], bf16)   # K.T
            v_sb = big.tile([P, T // P, DLOC], bf16)  # V natural (t on partitions)
            ctx_sb = big.tile([P, H_LOC, T], bf16)  # ctx.T

            def emit_body():
                emit_phase1()
                if interleave:
                    emit_phase2([0])
                    emit_phase3(range(0, T // P // 2))
                    emit_phase2([1])
                    emit_phase3(range(T // P // 2, T // P))
                else:
                    emit_phase2()
                    emit_phase3()

            # ---- phase 1: Q/K/V projections, streamed over token chunks --
            def emit_phase1():
              for c in range(CH):
                xc = xch.tile([P, KO, CHW], bf16, tag="xc", name="xc")
                nc.sync.dma_start(xc[:], xt_v[:, :, c * CHW:(c + 1) * CHW])

                for h in range(H_LOC):
                    hd = slice(h * P, (h + 1) * P)
                    psq = ps_tile()
                    for ko in range(KO):
                        nc.tensor.matmul(
                            psq[:], wq_sb[:, ko, hd], xc[:, ko, :],
                            start=(ko == 0), stop=(ko == KO - 1),
                        )
                    nc.scalar.activation(
                        qt_sb[:, h, c * CHW:(c + 1) * CHW], psq[:],
                        Ident, bias=bq_sb[:, h:h + 1],
                    )
                    psk = ps_tile()
                    for ko in range(KO):
                        nc.tensor.matmul(
                            psk[:], wk_sb[:, ko, hd], xc[:, ko, :],
                            start=(ko == 0), stop=(ko == KO - 1),
                        )
                    nc.scalar.activation(
                        kt_sb[:, h, c * CHW:(c + 1) * CHW], psk[:],
                        Ident, bias=bk_sb[:, h:h + 1],
                    )

                for tt in range(CHW // P):
                    psv = ps_tile()
                    for ko in range(KO):
                        nc.tensor.matmul(
                            psv[:, :DLOC], xc[:, ko, tt * P:(tt + 1) * P],
                            wv_sb[:, ko, :],
                            start=(ko == 0), stop=(ko == KO - 1),
                        )
                    nc.vector.tensor_copy(
                        v_sb[:, c * (CHW // P) + tt, :], psv[:, :DLOC]
                    )

            # ---- phase 2: attention per (batch, head) --------------------
            def emit_phase2(bs=tuple(range(B))):
              for b in bs:
                for h in range(H_LOC):
                    hd = slice(h * P, (h + 1) * P)
                    for qi in range(NQ):
                        qs = slice(b * S + qi * 512, b * S + (qi + 1) * 512)
                        ps_ctx = ps_tile()
                        ps_sum = ps_tile()
                        for kt in range(NKT):
                            ks = slice(b * S + kt * P, b * S + (kt + 1) * P)
                            ps_s = ps_tile()
                            nc.tensor.matmul(
                                ps_s[:], kt_sb[:, h, ks], qt_sb[:, h, qs],
                                start=True, stop=True,
                            )
                            pt = ptp.tile([P, 512], bf16, tag="pt", name="pt")
                            nc.scalar.activation(
                                pt[:], ps_s[:], Exp,
                                bias=mask_sb[:, b, kt:kt + 1], scale=SCALE,
                            )
                            if no_attn:
                                continue
                            nc.tensor.matmul(
                                ps_ctx[:], v_sb[:, b * NKT + kt, hd], pt[:],
                                start=(kt == 0), stop=(kt == NKT - 1),
                            )
                            if not no_sums:
                                nc.tensor.matmul(
                                    ps_sum[0:1, :], ones_sb[:], pt[:],
                                    start=(kt == 0), stop=(kt == NKT - 1),
                                )
                        if no_attn:
                            continue
                        if no_sums:
                            nc.vector.tensor_copy(ctx_sb[:, h, qs], ps_ctx[:])
                            continue
                        rcp = nrm.tile([1, 512], fp32, tag="rcp", name="rcp")
                        nc.vector.reciprocal(rcp[:], ps_sum[0:1, :])
                        rbc = nrm.tile([P, 512], fp32, tag="rbc", name="rbc")
                        if norm2:
                            # Drain the ctx psum to SBUF right away (frees the
                            # bank); the reciprocal broadcast (DRAM bounce)
                            # happens off the critical path.
                            ctxu = nrm.tile([P, 512], fp32, tag="ctxu", name="ctxu")
                            nc.vector.tensor_copy(ctxu[:], ps_ctx[:])
                            rdr = dscr.tile([1, 512], fp32, tag="rdr", name="rdr")
                            nc.sync.dma_start(rdr[:], rcp[:])
                            nc.sync.dma_start(rbc[:], rdr[:].to_broadcast((P, 512)))
                            nc.vector.tensor_mul(ctx_sb[:, h, qs], ctxu[:], rbc[:])
                        else:
                            rdr = dscr.tile([1, 512], fp32, tag="rdr", name="rdr")
                            nc.sync.dma_start(rdr[:], rcp[:])
                            nc.sync.dma_start(rbc[:], rdr[:].to_broadcast((P, 512)))
                            nc.vector.tensor_mul(ctx_sb[:, h, qs], ps_ctx[:], rbc[:])

            # ---- phase 3: partial output projection ----------------------
            def emit_phase3(tts=tuple(range(T // P))):
              if no_phase3:
                  return
              for tt in tts:
                ts_ = slice(tt * P, (tt + 1) * P)
                for oi in range(HIDDEN // 512):
                    os_ = slice(oi * 512, (oi + 1) * 512)
                    ps_o = ps_tile()
                    for h in range(H_LOC):
                        nc.tensor.matmul(
                            ps_o[:], ctx_sb[:, h, ts_], wo_sb[:, h, os_],
                            start=(h == 0), stop=(h == H_LOC - 1),
                        )
                    ob = obp.tile([P, 512], fp32, tag="ob", name="ob")
                    nc.vector.tensor_copy(ob[:], ps_o[:])
                    if not no_out_dma:
                        nc.sync.dma_start(out_d[ts_, os_], ob[:])

            if loop_k is None:
                emit_body()
            else:
                with tc.For_i(0, loop_k, 1):
                    emit_body()

    _split_multi_waits(nc)
    return nc


def _get_nc():
    if "nc" not in _CACHE:
        _CACHE["nc"] = _build_nc()
    return _CACHE["nc"]


def kernel(**inputs):
    hs = np.asarray(inputs["hidden_states"], dtype=np.float32)
    mask = np.asarray(inputs["attention_mask"], dtype=np.float32)
    Wq = np.asarray(inputs["Wq"], dtype=np.float32)
    bq = np.asarray(inputs["bq"], dtype=np.float32)
    Wk = np.asarray(inputs["Wk"], dtype=np.float32)
    bk = np.asarray(inputs["bk"], dtype=np.float32)
    Wv = np.asarray(inputs["Wv"], dtype=np.float32)
    bv = np.asarray(inputs["bv"], dtype=np.float32)
    Wo = np.asarray(inputs["Wo"], dtype=np.float32)
    bo = np.asarray(inputs["bo"], dtype=np.float32)

    x = hs.reshape(T, HIDDEN)
    xt = np.ascontiguousarray(x.T).astype(BF16NP)
    mask2 = np.ascontiguousarray(mask.reshape(B, S))

    in_maps = []
    for c in range(N_CORES):
        rs = slice(c * DLOC, (c + 1) * DLOC)
        in_maps.append({
            "xt": xt,
            "wqt": np.ascontiguousarray(Wq[rs, :].T).astype(BF16NP),
            "wkt": np.ascontiguousarray(Wk[rs, :].T).astype(BF16NP),
            "wvt": np.ascontiguousarray(Wv[rs, :].T).astype(BF16NP),
            "wot": np.ascontiguousarray(Wo[:, rs].T).astype(BF16NP),
            "bq": np.ascontiguousarray(bq[rs]),
            "bk": np.ascontiguousarray(bk[rs]),
            "mask": mask2,
        })

    from concourse.bass_utils import run_bass_kernel_spmd

    nc = _get_nc()
    trace = bool(int(os.environ.get("MHA_KERNEL_TRACE", "0")))
    res = run_bass_kernel_spmd(
        nc, in_maps, core_ids=list(range(N_CORES)), trace=trace,
        **({"trace_cores": list(range(N_CORES))} if trace else {}),
    )
    _CACHE["last_results"] = res

    out = np.sum(
        np.stack([r["out"] for r in res.results]), axis=0, dtype=np.float64
    )
    out += bv.astype(np.float64) @ Wo.T.astype(np.float64) + bo
    return out.astype(np.float32).reshape(B, S, HIDDEN)
